# revision 15
# baseline (speedup 1.0000x reference)
"""Causal self-attention (B=2, T=2048, C=1024, H=16) on 8 trn2 NeuronCores.

Sharding: core c = (batch b = c // 4, head-group g = c % 4). Each core
computes, for its batch, QKV for heads [4g, 4g+4), causal attention, and a
partial output projection through rows [256g, 256g+256) of W_proj. The host
sums the 4 partial projections per batch (tensor-parallel unshard) and adds
b_proj.

Per-core kernel structure (all matmul inputs bf16, fp32 PSUM):
  - qk^T is produced transposed ([channel, t]) so attention scores need no
    input transposes (contraction over d=64 sits on the partition axis).
  - Scores are computed TRANSPOSED (S^T[k, q] tiles): exp(S^T) is directly
    the P^T operand the PV matmul needs.
  - The two heads of a pair are row-packed per S^T matmul (K=64 at array
    rows 0/64) into one 2-bank PSUM tile, so one ACT op handles both exps.
  - Diagonal k-blocks narrow the score and PV matmuls to the causally valid
    q-range (the skipped PSUM/pt columns are never read); only the 128x128
    triangle block needs a 0/1 mask multiply (on the otherwise-idle GpSimd).
  - V carries an appended ones column (lhsT [128, 65]) so the PV matmul
    accumulates the softmax denominator as row 64 of y^T_aug for free.
  - PSUM (8 banks): scores 2x[128,1024] (4), PV accumulator 1x[65,1024]
    (2, both heads side by side -> one evacuation copy), fillers 2x[128,512].
  - The PV accumulator is evacuated by a single DVE copy immediately after
    the last PV matmul; normalization (reciprocal of the denominator row,
    DMA partition-broadcast of it, multiply into yT) runs off PSUM entirely
    and is deferred into the next iteration's filler slots so the DMA
    latency never head-blocks the in-order DVE queue.
  - QKV/V/projection matmul groups are emitted as ~2-matmul filler units
    BETWEEN attention steps (chunk qc+1's QKV computes while chunk qc's
    attention runs), keeping the PE dense so HAM stays at full clock.
  - All inputs are host-relaid so every load is one large-packet DMA.
  - y lands transposed ([d, q]), exactly the lhsT the projection needs;
    outputs are written bf16 (the host sums partials in fp32).
"""

import sys
from collections import deque

for _p in ("/opt/trn_rl_repo",):
    if _p not in sys.path:
        sys.path.insert(0, _p)

import numpy as np
import ml_dtypes

import concourse.bass as bass
import concourse.tile as tile
from concourse import bacc, mybir
from concourse.bass_utils import run_bass_kernel_spmd

BF16 = mybir.dt.bfloat16
F32 = mybir.dt.float32
NP_BF16 = ml_dtypes.bfloat16

B, T, C = 2, 2048, 1024
H, D = 16, 64
N_CORES = 8
CT = C // 128   # 8 contraction tiles
TQ = T // 128   # 16 key blocks
QC = T // 512   # 4 query chunks
SCALE = 1.0 / np.sqrt(D)

_compiled = None


def _build_nc(dbg=False):
    nc = bacc.Bacc("TRN2", target_bir_lowering=False, debug=False,
                   enable_asserts=False)
    if dbg:
        dbg_yT = nc.dram_tensor("dbg_yT", [128, 2, T], BF16, kind="ExternalOutput")

    xT_d = nc.dram_tensor("xT", [QC, 128, CT, 512], BF16, kind="ExternalInput")
    wqk_d = nc.dram_tensor("wqk", [4, 128, CT, 128], BF16, kind="ExternalInput")
    wv_d = nc.dram_tensor("wv", [128, CT, 256], BF16, kind="ExternalInput")
    wp_d = nc.dram_tensor("wp", [128, 2, C], BF16, kind="ExternalInput")
    bqk_d = nc.dram_tensor("bqk", [128, 4], F32, kind="ExternalInput")
    bv_d = nc.dram_tensor("bv", [128, 256], BF16, kind="ExternalInput")
    tri_d = nc.dram_tensor("tri", [128, 128], BF16, kind="ExternalInput")
    out_d = nc.dram_tensor("out", [TQ // 2, 128, 2048], BF16, kind="ExternalOutput")

    Exp = mybir.ActivationFunctionType.Exp

    with tile.TileContext(nc) as tc:
        with (
            tc.tile_pool(name="const", bufs=1) as cpool,
            tc.tile_pool(name="pt", bufs=5) as ptpool,
            tc.tile_pool(name="norm", bufs=2) as npool,
            tc.tile_pool(name="ostage", bufs=3) as opool,
            tc.tile_pool(name="scps", bufs=2, space="PSUM") as scps,
            tc.tile_pool(name="pvps", bufs=1, space="PSUM") as pvps,
            tc.tile_pool(name="mmps", bufs=2, space="PSUM") as mmps,
        ):
            # ---- persistent SBUF ----
            xT_s = cpool.tile([128, QC, CT, 512], BF16)
            wqk_s = cpool.tile([128, 4, CT, 128], BF16)
            wv_s = cpool.tile([128, CT, 256], BF16)
            wp_s = cpool.tile([128, 2, C], BF16)
            bqk_s = cpool.tile([128, 4], F32)
            bv_s = cpool.tile([128, 256], BF16)
            tri_s = cpool.tile([128, 128], BF16)

            warm = cpool.tile([128, 1], F32)
            nc.vector.memset(warm[:], 0.0)
            nc.scalar.activation(warm[:], warm[:], Exp)

            def dma_chunk(t4):
                nc.sync.dma_start(out=xT_s[:, t4], in_=xT_d.ap()[t4])

            dma_chunk(0)
            for j in (0, 2, 1, 3):
                nc.sync.dma_start(out=wqk_s[:, j], in_=wqk_d.ap()[j])
            nc.sync.dma_start(out=bqk_s[:], in_=bqk_d.ap()[:])
            nc.sync.dma_start(out=tri_s[:], in_=tri_d.ap()[:])
            nc.sync.dma_start(out=wv_s[:], in_=wv_d.ap()[:])
            nc.sync.dma_start(out=bv_s[:], in_=bv_d.ap()[:])
            nc.sync.dma_start(out=wp_s[:], in_=wp_d.ap()[:])

            qkT_s = cpool.tile([128, 4, T], BF16)
            v_s = cpool.tile([128, TQ, 4, 65], BF16)
            nc.vector.memset(v_s[:, :, :, 64:65], 1.0)
            yT_s = cpool.tile([128, 2, T], BF16)

            # ---- emission helpers (work split into ~2-matmul filler units) --
            fillers = deque()

            def emit_filler(n=1):
                for _ in range(n):
                    if fillers:
                        fillers.popleft()()

            def qkv_units(j, t4):
                # j 0/1: Q head-pairs {0,1}/{2,3}; j 2/3: K pairs {0,1}/{2,3}
                cell = {}

                def half(h0, j=j, t4=t4, cell=cell):
                    if h0 == 0:
                        cell["ps"] = mmps.tile([128, 512], F32, tag="mm", name="qkvps")
                    ps = cell["ps"]
                    for i in range(h0, h0 + 4):
                        nc.tensor.matmul(
                            ps[:],
                            wqk_s[:, j, i, :],
                            xT_s[:, t4, i, :],
                            start=(i == 0), stop=(i == CT - 1),
                        )
                    if h0 == 4:
                        nc.vector.tensor_scalar_add(
                            qkT_s[:, j, 512 * t4:512 * (t4 + 1)], ps[:],
                            bqk_s[:, j:j + 1])

                fillers.append(lambda: half(0))
                fillers.append(lambda: half(4))

            def v_unit(t):
                def run(t=t):
                    ps = mmps.tile([128, 256], F32, tag="mm")
                    t4, ti = t // 4, t % 4
                    for i in range(CT):
                        nc.tensor.matmul(
                            ps[:],
                            xT_s[:, t4, i, 128 * ti:128 * (ti + 1)],
                            wv_s[:, i, :],
                            start=(i == 0), stop=(i == CT - 1),
                        )
                    nc.vector.tensor_add(
                        v_s[:, t, :, 0:64],
                        ps[:].rearrange("p (h d) -> p h d", h=4),
                        bv_s[:].rearrange("p (h d) -> p h d", h=4))
                fillers.append(run)

            proj_cells = {}

            def proj_units(t):
                # t-tiles are handled in pairs (2tt, 2tt+1) -> one bf16 out DMA
                tt, u = t // 2, t % 2
                cell = proj_cells.setdefault(tt, {})

                def nhalf(n, t=t, u=u, cell=cell):
                    if u == 0 and n == 0:
                        cell["o"] = opool.tile([128, 2048], BF16, tag="o", name="ot")
                    o_t = cell["o"]
                    ps = mmps.tile([128, 512], F32, tag="mm")
                    for p2 in range(2):
                        nc.tensor.matmul(
                            ps[:],
                            yT_s[:, p2, 128 * t:128 * (t + 1)],
                            wp_s[:, p2, 512 * n:512 * (n + 1)],
                            start=(p2 == 0), stop=(p2 == 1),
                        )
                    dst = o_t[:, 1024 * u + 512 * n:1024 * u + 512 * (n + 1)]
                    if n == 0:
                        nc.scalar.copy(dst, ps[:])
                    else:
                        nc.vector.tensor_copy(dst, ps[:])

                def store(tt=tt, cell=cell):
                    nc.sync.dma_start(out=out_d.ap()[tt], in_=cell["o"][:])

                fillers.append(lambda: nhalf(0))
                fillers.append(lambda: nhalf(1))
                if u == 1:
                    fillers.append(store)
                return cell

            # ---- prologue: pair-0's Q/K for chunk 0 (inline), V as fillers
            # (the inner loop pops fillers BEFORE each consume, so v_unit(t)
            # is always emitted ahead of the consume that reads it) ----
            for j in (0, 2):
                qkv_units(j, 0)
            emit_filler(4)
            for t in range(4):
                v_unit(t)
            for j in (1, 3):
                qkv_units(j, 0)

            # ---- attention: S^T tiles [k-block, q-chunk], flash over k ----
            for qc in range(QC):
                if qc + 1 < QC:
                    fillers.append(lambda t4=qc + 1: dma_chunk(t4))
                    for j in (0, 2, 1, 3):
                        qkv_units(j, qc + 1)
                    for t in range(4 * (qc + 1), 4 * (qc + 2)):
                        v_unit(t)
                if qc >= 1:
                    for t in range(4 * (qc - 1), 4 * qc):
                        proj_units(t)

                for p in range(2):
                    jq, jk = p, 2 + p
                    nkb = 4 * qc + 4
                    y_ps = pvps.tile([65, 1024], F32, tag="pv")
                    pts = {}

                    def stage(kb, qc=qc, jq=jq, jk=jk, pts=None):
                        """score matmuls + exp (+ causal triangle mask)"""
                        m = kb - 4 * qc  # >= 0 on the diagonal chunk
                        off = 128 * m if m > 0 else 0
                        s_ps = scps.tile([128, 1024], F32, tag="sc")
                        for hi, part in ((0, slice(0, 64)), (1, slice(64, 128))):
                            nc.tensor.matmul(
                                s_ps[:, 512 * hi + off:512 * (hi + 1)],
                                qkT_s[part, jk, 128 * kb:128 * (kb + 1)],
                                qkT_s[part, jq, 512 * qc + off:512 * (qc + 1)],
                                start=True, stop=True,
                                tile_position=(64 * hi, 0), skip_group_check=True)
                        pt = ptpool.tile([128, 1024], BF16, tag="pt")
                        if off == 0:
                            nc.scalar.activation(pt[:], s_ps[:], Exp, scale=SCALE)
                        else:
                            for hi in range(2):
                                nc.scalar.activation(
                                    pt[:, 512 * hi + off:512 * (hi + 1)],
                                    s_ps[:, 512 * hi + off:512 * (hi + 1)],
                                    Exp, scale=SCALE)
                        if m >= 0:
                            for hi in range(2):
                                nc.gpsimd.tensor_mul(
                                    pt[:, 512 * hi + off:512 * hi + off + 128],
                                    pt[:, 512 * hi + off:512 * hi + off + 128],
                                    tri_s[:])
                        pts[kb] = pt

                    def consume(kb, p=p, qc=qc, y_ps=y_ps, nkb=nkb, pts=None):
                        pt = pts.pop(kb)
                        m = kb - 4 * qc
                        off = 128 * m if m > 0 else 0
                        for hi in range(2):
                            nc.tensor.matmul(
                                y_ps[:, 512 * hi + off:512 * (hi + 1)],
                                v_s[:, kb, 2 * p + hi, :],
                                pt[:, 512 * hi + off:512 * (hi + 1)],
                                start=(kb == 0), stop=(kb == nkb - 1))

                    DEPTH = 2
                    for kb in range(min(DEPTH, nkb)):
                        stage(kb, pts=pts)
                    for kb in range(nkb):
                        if kb + DEPTH < nkb:
                            stage(kb + DEPTH, pts=pts)
                        emit_filler(2)
                        consume(kb, pts=pts)

                    # evacuate the PV accumulator NOW (frees its PSUM banks);
                    # normalization runs off SBUF in upcoming filler slots.
                    yc = npool.tile([65, 1024], F32, tag="yc")
                    nc.vector.tensor_copy(yc[:], y_ps[:])
                    lb = npool.tile([64, 1024], F32, tag="lb")
                    nc.sync.dma_start(
                        out=lb[:],
                        in_=yc[64:65, :].unsqueeze(1).broadcast_to([1, 64, 1024]))

                    def norm_mul(p=p, qc=qc, yc=yc, lb=lb):
                        rb = npool.tile([64, 1024], F32, tag="rb")
                        nc.vector.reciprocal_approx_fast(rb[:], lb[:])
                        for hi in range(2):
                            nc.vector.tensor_mul(
                                yT_s[64 * hi:64 * (hi + 1), p,
                                     512 * qc:512 * (qc + 1)],
                                yc[0:64, 512 * hi:512 * (hi + 1)],
                                rb[:, 512 * hi:512 * (hi + 1)])

                    fillers.insert(min(9, len(fillers)), norm_mul)

            if dbg:
                nc.sync.dma_start(out=dbg_yT.ap()[:], in_=yT_s[:])

            # ---- epilogue: leftover fillers + final projection chunk ----
            emit_filler(len(fillers))
            for t in range(4 * (QC - 1), TQ):
                proj_units(t)
            emit_filler(len(fillers))

    nc.compile()
    return nc


def _shard_inputs(x, W_attn, b_attn, W_proj, b_proj):
    """Build the 8 per-core input maps (numpy, bf16 where applicable)."""
    # tri[p, j]: 0/1 keep-mask for a diagonal 128x128 S^T block: keep p <= j.
    pp = np.arange(128)[:, None]
    jj = np.arange(128)[None, :]
    tri = np.where(pp > jj, 0.0, 1.0).astype(NP_BF16)
    in_maps = []
    for c in range(N_CORES):
        b, g = c // 4, c % 4
        ch = slice(256 * g, 256 * (g + 1))
        wq = W_attn[:, ch]
        wk = W_attn[:, C:][:, ch]
        wv = W_attn[:, 2 * C:][:, ch]
        # wqk[j, p, i, :]: j-tile j's 128 output cols, contraction tile i
        wqk = np.concatenate([wq, wk], axis=1).astype(NP_BF16)  # [C, 512]
        wqk = np.ascontiguousarray(
            wqk.reshape(CT, 128, 4, 128).transpose(2, 1, 0, 3))
        bq = b_attn[ch]
        bk = b_attn[C:][ch]
        bv = b_attn[2 * C:][ch]
        bqk = np.concatenate([bq, bk]).reshape(4, 128).T.astype(np.float32)  # [128, 4]
        # xT[qc, p, i, tt] = x[b][512 qc + tt, 128 i + p]
        xTc = np.ascontiguousarray(
            x[b].T.reshape(CT, 128, QC, 512).transpose(2, 1, 0, 3)).astype(NP_BF16)
        wvc = np.ascontiguousarray(
            wv.astype(NP_BF16).reshape(CT, 128, 256).transpose(1, 0, 2))
        wpc = np.ascontiguousarray(
            W_proj[ch, :].astype(NP_BF16).reshape(2, 128, C).transpose(1, 0, 2))
        in_maps.append({
            "xT": xTc,
            "wqk": wqk,
            "wv": wvc,
            "wp": wpc,
            "bqk": np.ascontiguousarray(bqk),
            "bv": np.broadcast_to(bv.astype(NP_BF16), (128, 256)).copy(),
            "tri": tri,
        })
    return in_maps


def _run(in_maps, trace=False, **kw):
    global _compiled
    if _compiled is None:
        _compiled = _build_nc()
    return run_bass_kernel_spmd(_compiled, in_maps, list(range(N_CORES)),
                                trace=trace, **kw)


def kernel(x, W_attn, b_attn, W_proj, b_proj):
    x = np.asarray(x, dtype=np.float32)
    W_attn = np.asarray(W_attn, dtype=np.float32)
    b_attn = np.asarray(b_attn, dtype=np.float32)
    W_proj = np.asarray(W_proj, dtype=np.float32)
    b_proj = np.asarray(b_proj, dtype=np.float32)

    in_maps = _shard_inputs(x, W_attn, b_attn, W_proj, b_proj)
    res = _run(in_maps)
    out = np.zeros((B, T, C), dtype=np.float32)
    for c in range(N_CORES):
        # out[tt, p, 1024 u + cc] = partial[256 tt + 128 u + p, cc]
        o = np.asarray(res.results[c]["out"], dtype=np.float32)
        out[c // 4] += o.reshape(TQ // 2, 128, 2, C).transpose(
            0, 2, 1, 3).reshape(T, C)
    out += b_proj
    return out


# revision 17
# speedup vs baseline: 1.0996x; 1.0996x over previous
"""Causal self-attention (B=2, T=2048, C=1024, H=16) on 8 trn2 NeuronCores.

Sharding: core c = (batch b = c // 4, head-group g = c % 4). Each core
computes, for its batch, QKV for heads [4g, 4g+4), causal attention, and a
partial output projection through rows [256g, 256g+256) of W_proj. The host
sums the 4 partial projections per batch (tensor-parallel unshard) and adds
b_proj.

Per-core kernel structure (all matmul inputs bf16, fp32 PSUM):
  - qk^T is produced transposed ([channel, t]) so attention scores need no
    input transposes (contraction over d=64 sits on the partition axis).
  - Scores are computed TRANSPOSED (S^T[k, q] tiles): exp(S^T) is directly
    the P^T operand the PV matmul needs.
  - The two heads of a pair are row-packed per S^T matmul (K=64 at array
    rows 0/64) into one 2-bank PSUM tile, so one ACT op handles both exps.
  - Diagonal k-blocks narrow the score and PV matmuls to the causally valid
    q-range (the skipped PSUM/pt columns are never read); only the 128x128
    triangle block needs a 0/1 mask multiply (on the otherwise-idle GpSimd).
  - V carries an appended ones column (lhsT [128, 65]) so the PV matmul
    accumulates the softmax denominator as row 64 of y^T_aug for free.
  - PSUM (8 banks): scores 2x[128,1024] (4), PV accumulator 1x[65,1024]
    (2, both heads side by side -> one evacuation copy), fillers 2x[128,512].
  - The PV accumulator is evacuated by a single DVE copy immediately after
    the last PV matmul; normalization (reciprocal of the denominator row,
    DMA partition-broadcast of it, multiply into yT) runs off PSUM entirely
    and is deferred into the next iteration's filler slots so the DMA
    latency never head-blocks the in-order DVE queue.
  - QKV/V/projection matmul groups are emitted as ~2-matmul filler units
    BETWEEN attention steps (chunk qc+1's QKV computes while chunk qc's
    attention runs), keeping the PE dense so HAM stays at full clock.
  - All inputs are host-relaid so every load is one large-packet DMA.
  - y lands transposed ([d, q]), exactly the lhsT the projection needs;
    outputs are written bf16 (the host sums partials in fp32).
"""

import sys
from collections import deque

for _p in ("/opt/trn_rl_repo",):
    if _p not in sys.path:
        sys.path.insert(0, _p)

import numpy as np
import ml_dtypes

import concourse.bass as bass
import concourse.tile as tile
from concourse import bacc, mybir
from concourse.bass_utils import run_bass_kernel_spmd

BF16 = mybir.dt.bfloat16
F32 = mybir.dt.float32
NP_BF16 = ml_dtypes.bfloat16

B, T, C = 2, 2048, 1024
H, D = 16, 64
N_CORES = 8
CT = C // 128   # 8 contraction tiles
TQ = T // 128   # 16 key blocks
QC = T // 512   # 4 query chunks
SCALE = 1.0 / np.sqrt(D)

_compiled = None


def _build_nc(dbg=False):
    nc = bacc.Bacc("TRN2", target_bir_lowering=False, debug=False,
                   enable_asserts=False)
    if dbg:
        dbg_yT = nc.dram_tensor("dbg_yT", [128, 2, T], BF16, kind="ExternalOutput")

    xT_d = nc.dram_tensor("xT", [QC, 128, CT, 512], BF16, kind="ExternalInput")
    wqk_d = nc.dram_tensor("wqk", [4, 128, CT, 128], BF16, kind="ExternalInput")
    wv_d = nc.dram_tensor("wv", [128, CT, 256], BF16, kind="ExternalInput")
    wp_d = nc.dram_tensor("wp", [128, 2, C], BF16, kind="ExternalInput")
    bqk_d = nc.dram_tensor("bqk", [128, 4], F32, kind="ExternalInput")
    bv_d = nc.dram_tensor("bv", [128, 256], BF16, kind="ExternalInput")
    tri_d = nc.dram_tensor("tri", [128, 128], BF16, kind="ExternalInput")
    out_d = nc.dram_tensor("out", [TQ // 2, 128, 2048], BF16, kind="ExternalOutput")

    Exp = mybir.ActivationFunctionType.Exp

    with tile.TileContext(nc) as tc:
        with (
            tc.tile_pool(name="const", bufs=1) as cpool,
            tc.tile_pool(name="pt", bufs=5) as ptpool,
            tc.tile_pool(name="norm", bufs=2) as npool,
            tc.tile_pool(name="ostage", bufs=3) as opool,
            tc.tile_pool(name="scps", bufs=2, space="PSUM") as scps,
            tc.tile_pool(name="pvps", bufs=1, space="PSUM") as pvps,
            tc.tile_pool(name="mmps", bufs=2, space="PSUM") as mmps,
        ):
            # ---- persistent SBUF ----
            xT_s = cpool.tile([128, QC, CT, 512], BF16)
            wqk_s = cpool.tile([128, 4, CT, 128], BF16)
            wv_s = cpool.tile([128, CT, 256], BF16)
            wp_s = cpool.tile([128, 2, C], BF16)
            bqk_s = cpool.tile([128, 4], F32)
            bv_s = cpool.tile([128, 256], BF16)
            tri_s = cpool.tile([128, 128], BF16)

            warm = cpool.tile([128, 1], F32)
            nc.vector.memset(warm[:], 0.0)
            nc.scalar.activation(warm[:], warm[:], Exp)

            def dma_chunk(t4):
                nc.sync.dma_start(out=xT_s[:, t4], in_=xT_d.ap()[t4])

            dma_chunk(0)
            for j in (0, 2, 1, 3):
                nc.sync.dma_start(out=wqk_s[:, j], in_=wqk_d.ap()[j])
            nc.sync.dma_start(out=bqk_s[:], in_=bqk_d.ap()[:])
            nc.sync.dma_start(out=tri_s[:], in_=tri_d.ap()[:])
            nc.sync.dma_start(out=wv_s[:], in_=wv_d.ap()[:])
            nc.sync.dma_start(out=bv_s[:], in_=bv_d.ap()[:])
            nc.sync.dma_start(out=wp_s[:], in_=wp_d.ap()[:])

            qkT_s = cpool.tile([128, 4, T], BF16)
            v_s = cpool.tile([128, TQ, 4, 65], BF16)
            nc.vector.memset(v_s[:, :, :, 64:65], 1.0)
            yT_s = cpool.tile([128, 2, T], BF16)

            # ---- emission helpers (work split into ~2-matmul filler units) --
            fillers = deque()
            delayed = []   # (due_cycle, fn): emitted once the global attention
            cyc = [0]      # cycle counter passes due_cycle (so DVE ops that
                           # wait on DMAs never head-block the in-order queue)

            def emit_filler(n=1):
                due = [it for it in delayed if it[0] <= cyc[0]]
                for it in due:
                    delayed.remove(it)
                    it[1]()
                for _ in range(n):
                    if fillers:
                        fillers.popleft()()

            def qkv_units(j, t4):
                # j 0/1: Q head-pairs {0,1}/{2,3}; j 2/3: K pairs {0,1}/{2,3}
                cell = {}

                def half(h0, j=j, t4=t4, cell=cell):
                    if h0 == 0:
                        cell["ps"] = mmps.tile([128, 512], F32, tag="mm", name="qkvps")
                    ps = cell["ps"]
                    for i in range(h0, h0 + 4):
                        nc.tensor.matmul(
                            ps[:],
                            wqk_s[:, j, i, :],
                            xT_s[:, t4, i, :],
                            start=(i == 0), stop=(i == CT - 1),
                        )
                    if h0 == 4:
                        nc.vector.tensor_scalar_add(
                            qkT_s[:, j, 512 * t4:512 * (t4 + 1)], ps[:],
                            bqk_s[:, j:j + 1])

                fillers.append(lambda: half(0))
                fillers.append(lambda: half(4))

            def v_unit(t):
                def run(t=t):
                    ps = mmps.tile([128, 256], F32, tag="mm")
                    t4, ti = t // 4, t % 4
                    for i in range(CT):
                        nc.tensor.matmul(
                            ps[:],
                            xT_s[:, t4, i, 128 * ti:128 * (ti + 1)],
                            wv_s[:, i, :],
                            start=(i == 0), stop=(i == CT - 1),
                        )
                    nc.vector.tensor_add(
                        v_s[:, t, :, 0:64],
                        ps[:].rearrange("p (h d) -> p h d", h=4),
                        bv_s[:].rearrange("p (h d) -> p h d", h=4))
                fillers.append(run)

            proj_cells = {}

            def proj_units(t, into=None):
                # t-tiles are handled in pairs (2tt, 2tt+1) -> one bf16 out DMA
                if into is None:
                    into = fillers
                tt, u = t // 2, t % 2
                cell = proj_cells.setdefault(tt, {})

                def nhalf(n, t=t, u=u, cell=cell):
                    if u == 0 and n == 0:
                        cell["o"] = opool.tile([128, 2048], BF16, tag="o", name="ot")
                    o_t = cell["o"]
                    ps = mmps.tile([128, 512], F32, tag="mm")
                    for p2 in range(2):
                        nc.tensor.matmul(
                            ps[:],
                            yT_s[:, p2, 128 * t:128 * (t + 1)],
                            wp_s[:, p2, 512 * n:512 * (n + 1)],
                            start=(p2 == 0), stop=(p2 == 1),
                        )
                    dst = o_t[:, 1024 * u + 512 * n:1024 * u + 512 * (n + 1)]
                    if n == 0:
                        nc.scalar.copy(dst, ps[:])
                    else:
                        nc.vector.tensor_copy(dst, ps[:])

                def store(tt=tt, cell=cell):
                    nc.gpsimd.dma_start(out=out_d.ap()[tt], in_=cell["o"][:])

                into.append(lambda: nhalf(0))
                into.append(lambda: nhalf(1))
                if u == 1:
                    into.append(store)
                return cell

            # ---- prologue: pair-0's Q/K for chunk 0 (inline), V as fillers
            # (the inner loop pops fillers BEFORE each consume, so v_unit(t)
            # is always emitted ahead of the consume that reads it) ----
            for j in (0, 2):
                qkv_units(j, 0)
            emit_filler(4)
            for t in range(4):
                v_unit(t)
            for j in (1, 3):
                qkv_units(j, 0)

            # ---- attention: S^T tiles [k-block, q-chunk], flash over k ----
            for qc in range(QC):
                if qc + 1 < QC:
                    fillers.append(lambda t4=qc + 1: dma_chunk(t4))
                    for j in (0, 2, 1, 3):
                        qkv_units(j, qc + 1)
                    for t in range(4 * (qc + 1), 4 * (qc + 2)):
                        v_unit(t)
                if qc >= 1:
                    # gated behind the (qc-1, p1) normalization flush (+5)
                    lst = []
                    for t in range(4 * (qc - 1), 4 * qc):
                        proj_units(t, into=lst)
                    delayed.append(
                        (cyc[0] + 6, lambda lst=lst: fillers.extend(lst)))

                for p in range(2):
                    jq, jk = p, 2 + p
                    nkb = 4 * qc + 4
                    y_ps = pvps.tile([65, 1024], F32, tag="pv")
                    pts = {}

                    def stage(kb, qc=qc, jq=jq, jk=jk, pts=None):
                        """score matmuls + exp (+ causal triangle mask)"""
                        m = kb - 4 * qc  # >= 0 on the diagonal chunk
                        off = 128 * m if m > 0 else 0
                        s_ps = scps.tile([128, 1024], F32, tag="sc")
                        for hi, part in ((0, slice(0, 64)), (1, slice(64, 128))):
                            nc.tensor.matmul(
                                s_ps[:, 512 * hi + off:512 * (hi + 1)],
                                qkT_s[part, jk, 128 * kb:128 * (kb + 1)],
                                qkT_s[part, jq, 512 * qc + off:512 * (qc + 1)],
                                start=True, stop=True,
                                tile_position=(64 * hi, 0), skip_group_check=True)
                        pt = ptpool.tile([128, 1024], BF16, tag="pt")
                        if off == 0:
                            nc.scalar.activation(pt[:], s_ps[:], Exp, scale=SCALE)
                        else:
                            for hi in range(2):
                                nc.scalar.activation(
                                    pt[:, 512 * hi + off:512 * (hi + 1)],
                                    s_ps[:, 512 * hi + off:512 * (hi + 1)],
                                    Exp, scale=SCALE)
                        if m >= 0:
                            for hi in range(2):
                                nc.gpsimd.tensor_mul(
                                    pt[:, 512 * hi + off:512 * hi + off + 128],
                                    pt[:, 512 * hi + off:512 * hi + off + 128],
                                    tri_s[:])
                        pts[kb] = pt

                    def consume(kb, p=p, qc=qc, y_ps=y_ps, nkb=nkb, pts=None):
                        pt = pts.pop(kb)
                        m = kb - 4 * qc
                        off = 128 * m if m > 0 else 0
                        for hi in range(2):
                            nc.tensor.matmul(
                                y_ps[:, 512 * hi + off:512 * (hi + 1)],
                                v_s[:, kb, 2 * p + hi, :],
                                pt[:, 512 * hi + off:512 * (hi + 1)],
                                start=(kb == 0), stop=(kb == nkb - 1))

                    DEPTH = 2
                    for kb in range(min(DEPTH, nkb)):
                        stage(kb, pts=pts)
                    for kb in range(nkb):
                        if kb + DEPTH < nkb:
                            stage(kb + DEPTH, pts=pts)
                        cyc[0] += 1
                        emit_filler(2)
                        consume(kb, pts=pts)

                    # evacuate the PV accumulator NOW (frees its PSUM banks);
                    # normalization runs off SBUF in upcoming filler slots.
                    yc = npool.tile([65, 1024], F32, tag="yc")
                    nc.vector.tensor_copy(yc[:], y_ps[:])
                    lb = npool.tile([64, 1024], F32, tag="lb")
                    nc.sync.dma_start(
                        out=lb[:],
                        in_=yc[64:65, :].unsqueeze(1).broadcast_to([1, 64, 1024]))

                    def norm_mul(p=p, qc=qc, yc=yc, lb=lb):
                        rb = npool.tile([64, 1024], F32, tag="rb")
                        nc.vector.reciprocal_approx_fast(rb[:], lb[:])
                        for hi in range(2):
                            nc.vector.tensor_mul(
                                yT_s[64 * hi:64 * (hi + 1), p,
                                     512 * qc:512 * (qc + 1)],
                                yc[0:64, 512 * hi:512 * (hi + 1)],
                                rb[:, 512 * hi:512 * (hi + 1)])

                    delayed.append((cyc[0] + 5, norm_mul))

            if dbg:
                nc.sync.dma_start(out=dbg_yT.ap()[:], in_=yT_s[:])

            # ---- epilogue: leftover fillers + final projection chunk ----
            emit_filler(len(fillers))
            for due, fn in sorted(delayed):
                fn()
            delayed.clear()
            for t in range(4 * (QC - 1), TQ):
                proj_units(t)
            emit_filler(len(fillers))

    nc.compile()
    return nc


def _shard_inputs(x, W_attn, b_attn, W_proj, b_proj):
    """Build the 8 per-core input maps (numpy, bf16 where applicable)."""
    # tri[p, j]: 0/1 keep-mask for a diagonal 128x128 S^T block: keep p <= j.
    pp = np.arange(128)[:, None]
    jj = np.arange(128)[None, :]
    tri = np.where(pp > jj, 0.0, 1.0).astype(NP_BF16)
    in_maps = []
    for c in range(N_CORES):
        b, g = c // 4, c % 4
        ch = slice(256 * g, 256 * (g + 1))
        wq = W_attn[:, ch]
        wk = W_attn[:, C:][:, ch]
        wv = W_attn[:, 2 * C:][:, ch]
        # wqk[j, p, i, :]: j-tile j's 128 output cols, contraction tile i
        wqk = np.concatenate([wq, wk], axis=1).astype(NP_BF16)  # [C, 512]
        wqk = np.ascontiguousarray(
            wqk.reshape(CT, 128, 4, 128).transpose(2, 1, 0, 3))
        bq = b_attn[ch]
        bk = b_attn[C:][ch]
        bv = b_attn[2 * C:][ch]
        bqk = np.concatenate([bq, bk]).reshape(4, 128).T.astype(np.float32)  # [128, 4]
        # xT[qc, p, i, tt] = x[b][512 qc + tt, 128 i + p]
        xTc = np.ascontiguousarray(
            x[b].T.reshape(CT, 128, QC, 512).transpose(2, 1, 0, 3)).astype(NP_BF16)
        wvc = np.ascontiguousarray(
            wv.astype(NP_BF16).reshape(CT, 128, 256).transpose(1, 0, 2))
        wpc = np.ascontiguousarray(
            W_proj[ch, :].astype(NP_BF16).reshape(2, 128, C).transpose(1, 0, 2))
        in_maps.append({
            "xT": xTc,
            "wqk": wqk,
            "wv": wvc,
            "wp": wpc,
            "bqk": np.ascontiguousarray(bqk),
            "bv": np.broadcast_to(bv.astype(NP_BF16), (128, 256)).copy(),
            "tri": tri,
        })
    return in_maps


def _run(in_maps, trace=False, **kw):
    global _compiled
    if _compiled is None:
        _compiled = _build_nc()
    return run_bass_kernel_spmd(_compiled, in_maps, list(range(N_CORES)),
                                trace=trace, **kw)


def kernel(x, W_attn, b_attn, W_proj, b_proj):
    x = np.asarray(x, dtype=np.float32)
    W_attn = np.asarray(W_attn, dtype=np.float32)
    b_attn = np.asarray(b_attn, dtype=np.float32)
    W_proj = np.asarray(W_proj, dtype=np.float32)
    b_proj = np.asarray(b_proj, dtype=np.float32)

    in_maps = _shard_inputs(x, W_attn, b_attn, W_proj, b_proj)
    res = _run(in_maps)
    out = np.zeros((B, T, C), dtype=np.float32)
    for c in range(N_CORES):
        # out[tt, p, 1024 u + cc] = partial[256 tt + 128 u + p, cc]
        o = np.asarray(res.results[c]["out"], dtype=np.float32)
        out[c // 4] += o.reshape(TQ // 2, 128, 2, C).transpose(
            0, 2, 1, 3).reshape(T, C)
    out += b_proj
    return out
